# revision 1
# baseline (speedup 1.0000x reference)
"""CBOW negative-sampling loss kernel for 8 Trainium2 NeuronCores.

The reference computes one-hot @ table matmuls (embedding lookups in
disguise) followed by a tiny log-sigmoid loss.  Device-side algorithm:

Phase A (index extraction, streaming):
  Every one-hot row (50000 wide) is laid out as 4 partitions x 12500.
  Stream chunks, multiply by an iota tile whose value at (p, j) is
  65536 + (p%4)*12500 + j on the vector engine, accumulate along free
  dim on the scalar engine.  A [128]->[32] fold matmul on the tensor
  engine sums each row's 4 quarters, giving val = 65536*cnt + idx
  exactly in fp32 (all quantities < 2^17, one-hot rows have <= one 1).

Phase B (gather + loss):
  cnt = (val >= 65536), idx = val - 65536*cnt.  Indices go to DRAM
  scratch in flat row order [vo(32) | vi(192) | neg(320)], are read
  back 128-rows-at-a-time, and drive single-offset indirect DMA
  gathers of U rows plus the per-row replicated vo V-row.  Per-row
  dots d = U_row . V_vo_row via DVE mult + ACT accumulate; then
  log-sigmoid terms via Exp/Log (one ACT table set) and per-batch
  reductions through small DRAM reshuffles.

Host: batch-shard across 8 cores, mean of the 256 per-batch terms.
"""
import numpy as np

import concourse.bass as bass
import concourse.mybir as mybir
from concourse.tile import TileContext
from concourse.bass_utils import run_bass_kernel_spmd

VOC = 50000
EMB = 300
B = 256
CTX = 6
K = 10
NCORES = 8
BPC = B // NCORES                    # 32 batch rows per core
NV = BPC * CTX                       # 192 vi rows per core
NN = BPC * K                         # 320 neg rows per core
NROWS = BPC + NV + NN                # 544 one-hot rows per core
NTILES = NROWS // 32                 # 17 extraction tiles of [128, 12500]
QW = VOC // 4                        # 12500 per partition-quarter
CH = QW // 2                         # 6250 free-dim chunk
NPAD = 640                           # padded flat row count (5 * 128)
NG = 5                               # gather tiles (4 full + 1 of 32 rows)
MARK = 65536.0                       # cnt marker (> max idx, power of 2)

F32 = mybir.dt.float32
I32 = mybir.dt.int32


def _split_multi_waits(nc):
    """This env's walrus accepts only ONE sync wait per instruction.
    Hoist extra waits into single-wait NoOps right before the owner."""
    cnt = 0
    for fn in nc.m.functions:
        for blk in fn.blocks:
            insts = list(blk.instructions)
            if not any(
                i.sync_info and i.sync_info.on_wait and len(i.sync_info.on_wait) > 1
                for i in insts
            ):
                continue
            new = []
            for inst in insts:
                si = inst.sync_info
                if si and si.on_wait and len(si.on_wait) > 1:
                    waits = list(si.on_wait)
                    for w in waits[:-1]:
                        cnt += 1
                        nop = mybir.InstNoOp(
                            name=f"mwsplit-{cnt}", engine=inst.engine, ins=[], outs=[]
                        )
                        nop.sync_info = mybir.SyncInfo(on_wait=[w], on_update=[])
                        new.append(nop)
                    inst.sync_info = mybir.SyncInfo(
                        on_wait=[waits[-1]], on_update=list(si.on_update or [])
                    )
                new.append(inst)
            blk.instructions = new
    return cnt


def _build():
    nc = bass.Bass(enable_partition_id=False)

    vo = nc.declare_dram_parameter("vo", [BPC, VOC], F32, isOutput=False)
    vi = nc.declare_dram_parameter("vi", [NV, VOC], F32, isOutput=False)
    ng = nc.declare_dram_parameter("ng", [NN, VOC], F32, isOutput=False)
    V = nc.declare_dram_parameter("V", [VOC, EMB], F32, isOutput=False)
    U = nc.declare_dram_parameter("U", [VOC, EMB], F32, isOutput=False)
    iota = nc.declare_dram_parameter("iota", [128, QW], F32, isOutput=False)
    foldq = nc.declare_dram_parameter("foldq", [128, 32], F32, isOutput=False)
    d_out = nc.declare_dram_parameter("dout", [128, NG], F32, isOutput=True)
    c_out = nc.declare_dram_parameter("cout", [32, NTILES], F32, isOutput=True)

    # per-tile [128, QW] sources: 4 partition-quarters per row
    srcs = [vo.rearrange("r (q f) -> (r q) f", q=4)]
    for u in range(CTX):
        srcs.append(vi[32 * u:32 * (u + 1), :].rearrange("r (q f) -> (r q) f", q=4))
    for u in range(K):
        srcs.append(ng[32 * u:32 * (u + 1), :].rearrange("r (q f) -> (r q) f", q=4))
    assert len(srcs) == NTILES

    with TileContext(nc) as tc:
        with (
            tc.tile_pool(name="const", bufs=1) as cpool,
            tc.tile_pool(name="data", bufs=3) as dpool,
            tc.tile_pool(name="prod", bufs=2) as ppool,
            tc.tile_pool(name="small", bufs=1) as spool,
            tc.tile_pool(name="gath", bufs=2) as gpool,
            tc.tile_pool(name="psum", bufs=1, space="PSUM") as psum_pool,
            tc.tile_pool(name="dram", bufs=1, space="DRAM") as dram_pool,
        ):
            iota_t = cpool.tile([128, QW], F32, tag="iota")
            nc.sync.dma_start(out=iota_t[:], in_=iota[:])
            foldq_t = cpool.tile([128, 32], F32, tag="foldq")
            nc.sync.dma_start(out=foldq_t[:], in_=foldq[:])

            # ---------------- Phase A: streaming extraction ----------------
            vals = spool.tile([128, NTILES * 2], F32, tag="vals")
            for t in range(NTILES):
                for h in range(2):
                    chunk = dpool.tile([128, CH], F32, tag="chunk")
                    nc.sync.dma_start(
                        out=chunk[:], in_=srcs[t][:, h * CH:(h + 1) * CH]
                    )
                    prod = ppool.tile([128, CH], F32, tag="prod")
                    nc.vector.tensor_tensor(
                        out=prod[:], in0=chunk[:],
                        in1=iota_t[:, h * CH:(h + 1) * CH],
                        op=mybir.AluOpType.mult,
                    )
                    col = 2 * t + h
                    nc.scalar.activation(
                        out=prod[:], in_=prod[:],
                        func=mybir.ActivationFunctionType.Copy,
                        accum_out=vals[:, col:col + 1],
                    )

            vals17 = spool.tile([128, NTILES], F32, tag="vals17")
            nc.vector.tensor_reduce(
                out=vals17[:], in_=vals[:].rearrange("p (t h) -> p t h", h=2),
                axis=mybir.AxisListType.X,
                op=mybir.AluOpType.add,
            )
            pvals = psum_pool.tile([32, NTILES], F32, tag="pvals")
            nc.tensor.matmul(
                out=pvals[:], lhsT=foldq_t[:], rhs=vals17[:], start=True, stop=True
            )
            # cnt = (val >= MARK), idx = val - MARK*cnt       [32, 17]
            cnt32 = spool.tile([32, NTILES], F32, tag="cnt32")
            nc.vector.tensor_scalar(
                out=cnt32[:], in0=pvals[:], scalar1=MARK, scalar2=None,
                op0=mybir.AluOpType.is_ge,
            )
            mk32 = spool.tile([32, NTILES], F32, tag="mk32")
            nc.vector.tensor_scalar(
                out=mk32[:], in0=cnt32[:], scalar1=MARK, scalar2=None,
                op0=mybir.AluOpType.mult,
            )
            idx32 = spool.tile([32, NTILES], F32, tag="idx32")
            nc.vector.tensor_tensor(
                out=idx32[:], in0=pvals[:], in1=mk32[:],
                op=mybir.AluOpType.subtract,
            )

            # ---------------- scratch round trips ----------------
            # flat row order: [vo(32) | vi(192) | neg(320)]; every DMA keeps
            # the DRAM-side innermost dim contiguous (DGE requirement here).
            sidx = dram_pool.tile([NROWS], F32, tag="sidx")
            svob = dram_pool.tile([NROWS], F32, tag="svob")
            for t in range(NTILES):
                nc.sync.dma_start(
                    out=sidx[32 * t:32 * (t + 1)].unsqueeze(1), in_=idx32[:, t:t + 1]
                )
            # voB offsets: vo idx replicated to every row's slot
            voidx = idx32[:, 0:1]
            rep6 = spool.tile([32, CTX], F32, tag="rep6")
            nc.vector.tensor_copy(out=rep6[:], in_=voidx.to_broadcast([32, CTX]))
            rep10 = spool.tile([32, K], F32, tag="rep10")
            nc.vector.tensor_copy(out=rep10[:], in_=voidx.to_broadcast([32, K]))
            nc.sync.dma_start(out=svob[:BPC].unsqueeze(1), in_=voidx)
            nc.sync.dma_start(
                out=svob[BPC:BPC + NV].rearrange("(b c) -> b c", c=CTX), in_=rep6[:]
            )
            nc.sync.dma_start(
                out=svob[BPC + NV:NROWS].rearrange("(b c) -> b c", c=K), in_=rep10[:]
            )

            # readbacks in gather-tile layout: rb[p, g] = flat[128 g + p]
            rb_idx = spool.tile([128, NG], F32, tag="rb_idx")
            rb_vob = spool.tile([128, NG], F32, tag="rb_vob")
            for g in range(NG):
                pg = 128 if g < NG - 1 else NROWS - 128 * (NG - 1)
                s = slice(128 * g, 128 * g + pg)
                nc.sync.dma_start(out=rb_idx[:pg, g:g + 1], in_=sidx[s].unsqueeze(1))
                nc.sync.dma_start(out=rb_vob[:pg, g:g + 1], in_=svob[s].unsqueeze(1))

            ofs_u = spool.tile([128, NG], I32, tag="ofs_u")
            nc.vector.tensor_copy(out=ofs_u[:], in_=rb_idx[:])
            ofs_v = spool.tile([128, NG], I32, tag="ofs_v")
            nc.vector.tensor_copy(out=ofs_v[:], in_=rb_vob[:])

            # ---------------- Phase B: gathers + per-row dots ----------------
            dall = spool.tile([128, NG], F32, tag="dall")
            nc.vector.memset(dall[:], 0.0)
            for g in range(NG):
                pg = 128 if g < NG - 1 else NROWS - 128 * (NG - 1)
                rowE = gpool.tile([128, EMB], F32, tag="rowE")
                nc.gpsimd.indirect_dma_start(
                    out=rowE[:pg, :], out_offset=None, in_=U[:],
                    in_offset=bass.IndirectOffsetOnAxis(ap=ofs_u[:pg, g:g + 1], axis=0),
                )
                voB = gpool.tile([128, EMB], F32, tag="voB")
                nc.gpsimd.indirect_dma_start(
                    out=voB[:pg, :], out_offset=None, in_=V[:],
                    in_offset=bass.IndirectOffsetOnAxis(ap=ofs_v[:pg, g:g + 1], axis=0),
                )
                prodB = gpool.tile([128, EMB], F32, tag="prodB")
                nc.vector.tensor_tensor(
                    out=prodB[:pg, :], in0=rowE[:pg, :], in1=voB[:pg, :],
                    op=mybir.AluOpType.mult,
                )
                nc.scalar.activation(
                    out=prodB[:pg, :], in_=prodB[:pg, :],
                    func=mybir.ActivationFunctionType.Copy,
                    accum_out=dall[:pg, g:g + 1],
                )

            nc.sync.dma_start(out=d_out[:], in_=dall[:])
            nc.sync.dma_start(out=c_out[:], in_=cnt32[:])

    _split_multi_waits(nc)
    return nc


def _consts():
    p = np.arange(128)
    iota_np = (
        MARK + (p % 4)[:, None] * QW + np.arange(QW)[None, :]
    ).astype(np.float32)
    foldq_np = np.zeros((128, 32), np.float32)
    foldq_np[p, p // 4] = 1.0
    return iota_np, foldq_np


_CACHE = {}


def kernel(vo, vi, neg_samples, V, U):
    if "nc" not in _CACHE:
        _CACHE["nc"] = _build()
        _CACHE["consts"] = _consts()
    nc = _CACHE["nc"]
    iota_np, foldq_np = _CACHE["consts"]

    vo = np.ascontiguousarray(vo, dtype=np.float32)
    vi = np.ascontiguousarray(vi, dtype=np.float32)
    neg = np.ascontiguousarray(neg_samples, dtype=np.float32)
    V = np.ascontiguousarray(V, dtype=np.float32)
    U = np.ascontiguousarray(U, dtype=np.float32)

    in_maps = []
    for c in range(NCORES):
        sl = slice(c * BPC, (c + 1) * BPC)
        in_maps.append({
            "vo": vo[sl],
            "vi": vi[sl].reshape(NV, VOC),
            "ng": neg[sl].reshape(NN, VOC),
            "V": V,
            "U": U,
            "iota": iota_np,
            "foldq": foldq_np,
        })

    res = run_bass_kernel_spmd(nc, in_maps, list(range(NCORES)))
    obs = []
    for r in res.results:
        d_flat = r["dout"].flatten(order="F")[:NROWS]
        c_flat = r["cout"].flatten(order="F")[:NROWS]
        d_vi = d_flat[BPC:BPC + NV].reshape(BPC, CTX)
        c_vi = c_flat[BPC:BPC + NV].reshape(BPC, CTX)
        d_ng = d_flat[BPC + NV:NROWS].reshape(BPC, K)
        lp = (d_vi * c_vi).sum(axis=1)
        ms = c_vi.sum(axis=1)
        x = lp / ms
        left = -np.log1p(np.exp(-x))
        right = (-np.log1p(np.exp(d_ng))).sum(axis=1)
        obs.append(-(left + right))
    ob = np.concatenate(obs)
    return np.float32(ob.mean(dtype=np.float64))



# revision 8
# speedup vs baseline: 1.0991x; 1.0991x over previous
"""CBOW negative-sampling loss kernel for 8 Trainium2 NeuronCores.

The reference computes one-hot @ table matmuls (embedding lookups in
disguise) followed by a tiny log-sigmoid loss.  Device-side algorithm:

Phase A (index extraction, streaming):
  Every one-hot row (50000 wide) is laid out as 4 partitions x 12500.
  The iota tile (value 65536 + (p%4)*12500 + j) is generated on-device
  (gpsimd iota + one DVE pass) instead of being DMAed from DRAM, so the
  HBM stream is exactly the one-hot bytes.  Each [128, 6250] chunk is
  consumed by ONE fused DVE tensor_tensor_reduce (mult + row-sum), and
  a per-quarter [128]->[128] fold matmul places each tile's 32 row
  values directly into a [128, col] PSUM layout (flat row = 128*col+p),
  eliminating the DRAM scratch round trips of the earlier version.

Phase B (gather + dots, pipelined per column):
  vo is tile 0 and owns psum column 0: its indices are ready early, so
  the 32 V rows are gathered and replicated to all 128 row slots of
  each column with tiny 0/1 matmuls on the (otherwise idle) tensor
  engine while streaming continues.  Each of the 4 vi/neg columns is
  finished as soon as its 4 extraction tiles land: idx = val - 65536*cnt,
  indirect-gather 128 U rows, fused DVE dot against the replicated V
  rows.  Only the last column's ~10us of work trails the final DMA.

Host: batch-shard across 8 cores, log-sigmoid loss terms + mean on CPU.
"""
import numpy as np

import concourse.bass as bass
import concourse.mybir as mybir
from concourse.tile import TileContext
from concourse.bass_utils import run_bass_kernel_spmd

VOC = 50000
EMB = 300
B = 256
CTX = 6
K = 10
NCORES = 8
BPC = B // NCORES                    # 32 batch rows per core
NV = BPC * CTX                       # 192 vi rows per core
NN = BPC * K                         # 320 neg rows per core
NTILES = 1 + (NV + NN) // 32         # 17 extraction tiles of [128, 12500]
QW = VOC // 4                        # 12500 per partition-quarter
CH = QW // 2                         # 6250 free-dim chunk
NCOL = 4                             # vi+neg columns (128 flat rows each)
MARK = 65536.0                       # cnt marker (> max idx, power of 2)

F32 = mybir.dt.float32
I32 = mybir.dt.int32


def _split_multi_waits(nc):
    """This env's walrus accepts only ONE sync wait per instruction.
    Hoist extra waits into single-wait NoOps right before the owner."""
    cnt = 0
    for fn in nc.m.functions:
        for blk in fn.blocks:
            insts = list(blk.instructions)
            if not any(
                i.sync_info and i.sync_info.on_wait and len(i.sync_info.on_wait) > 1
                for i in insts
            ):
                continue
            new = []
            for inst in insts:
                si = inst.sync_info
                if si and si.on_wait and len(si.on_wait) > 1:
                    waits = list(si.on_wait)
                    for w in waits[:-1]:
                        cnt += 1
                        nop = mybir.InstNoOp(
                            name=f"mwsplit-{cnt}", engine=inst.engine, ins=[], outs=[]
                        )
                        nop.sync_info = mybir.SyncInfo(on_wait=[w], on_update=[])
                        new.append(nop)
                    inst.sync_info = mybir.SyncInfo(
                        on_wait=[waits[-1]], on_update=list(si.on_update or [])
                    )
                new.append(inst)
            blk.instructions = new
    return cnt


def _build(split_waits=True):
    nc = bass.Bass(enable_partition_id=False)

    vo = nc.declare_dram_parameter("vo", [BPC, VOC], F32, isOutput=False)
    vi = nc.declare_dram_parameter("vi", [NV, VOC], F32, isOutput=False)
    ng = nc.declare_dram_parameter("ng", [NN, VOC], F32, isOutput=False)
    V = nc.declare_dram_parameter("V", [VOC, EMB], F32, isOutput=False)
    U = nc.declare_dram_parameter("U", [VOC, EMB], F32, isOutput=False)
    qoff = nc.declare_dram_parameter("qoff", [128, 1], F32, isOutput=False)
    foldq = nc.declare_dram_parameter("foldq", [128, 4 * 128], F32, isOutput=False)
    repmat = nc.declare_dram_parameter("repmat", [32, NCOL * 128], F32, isOutput=False)
    d_out = nc.declare_dram_parameter("out", [128, 2 * NCOL], F32, isOutput=True)

    # per-tile [128, QW] sources: 4 partition-quarters per row.
    # tile 0 = vo (owns psum column 0); tiles 1..16 = vi(6) + neg(10),
    # grouped 4 tiles -> one 128-row column (flat row f = 128*(c-1) + p).
    srcs = [vo.rearrange("r (q f) -> (r q) f", q=4)]
    for u in range(CTX):
        srcs.append(vi[32 * u:32 * (u + 1), :].rearrange("r (q f) -> (r q) f", q=4))
    for u in range(K):
        srcs.append(ng[32 * u:32 * (u + 1), :].rearrange("r (q f) -> (r q) f", q=4))
    assert len(srcs) == NTILES

    with TileContext(nc) as tc:
        with (
            tc.tile_pool(name="const", bufs=1) as cpool,
            tc.tile_pool(name="data", bufs=4) as dpool,
            tc.tile_pool(name="vals", bufs=3) as vpool,
            tc.tile_pool(name="small", bufs=1) as spool,
            tc.tile_pool(name="col", bufs=2) as lpool,
            tc.tile_pool(name="gath", bufs=2) as gpool,
            tc.tile_pool(name="quad", bufs=2, space="PSUM") as qpool,
            tc.tile_pool(name="repp", bufs=1, space="PSUM") as rpool,
            tc.tile_pool(name="vops", bufs=1, space="PSUM") as opool,
        ):
            # ---------------- constants (tiny DMAs + on-device iota) --------
            qoff_t = cpool.tile([128, 1], F32, tag="qoff")
            nc.sync.dma_start(out=qoff_t[:], in_=qoff[:])
            foldq_t = cpool.tile([128, 4 * 128], F32, tag="foldq")
            nc.sync.dma_start(out=foldq_t[:], in_=foldq[:])
            repmat_t = cpool.tile([32, NCOL * 128], F32, tag="repmat")
            nc.sync.dma_start(out=repmat_t[:], in_=repmat[:])

            iota_t = cpool.tile([128, QW], F32, tag="iota")
            nc.gpsimd.iota(
                out=iota_t[:], pattern=[[1, QW]], base=int(MARK),
                channel_multiplier=0, allow_small_or_imprecise_dtypes=True,
            )
            # iota[p, j] = 65536 + (p%4)*12500 + j
            nc.vector.tensor_scalar(
                out=iota_t[:], in0=iota_t[:], scalar1=qoff_t[:, 0:1], scalar2=None,
                op0=mybir.AluOpType.add,
            )

            out_t = spool.tile([128, 2 * NCOL], F32, tag="out_t")
            repVs = [None] * (NCOL + 1)

            # ---------------- streaming extraction + pipelined columns -----
            for t in range(NTILES):
                vt = vpool.tile([128, 2], F32, tag="vt")
                for h in range(2):
                    chunk = dpool.tile([128, CH], F32, tag="chunk")
                    nc.sync.dma_start(
                        out=chunk[:], in_=srcs[t][:, h * CH:(h + 1) * CH]
                    )
                    # fused: prod = (chunk * 1) * iota ; vt[:,h] = sum(prod)
                    nc.vector.scalar_tensor_tensor(
                        out=chunk[:], in0=chunk[:], scalar=1.0,
                        in1=iota_t[:, h * CH:(h + 1) * CH],
                        op0=mybir.AluOpType.mult, op1=mybir.AluOpType.mult,
                        accum_out=vt[:, h:h + 1],
                    )

                if t == 0:
                    # vo fold: quarters of row r -> partition r (cols 0..31
                    # of foldq block 0); psum col 0 closes immediately.
                    pvo = opool.tile([128, 2], F32, tag="pvo")
                    nc.tensor.matmul(
                        out=pvo[:], lhsT=foldq_t[:, 0:128], rhs=vt[:],
                        start=True, stop=True,
                    )
                    pvalv = spool.tile([128, 1], F32, tag="pvalv")
                    nc.vector.tensor_reduce(
                        out=pvalv[:], in_=pvo[:],
                        axis=mybir.AxisListType.X, op=mybir.AluOpType.add,
                    )
                    # vo rows always valid: idx = val - MARK
                    idxv = spool.tile([32, 1], F32, tag="idxv")
                    nc.vector.tensor_scalar(
                        out=idxv[:], in0=pvalv[0:32, :], scalar1=-MARK,
                        scalar2=None, op0=mybir.AluOpType.add,
                    )
                    ofsv = spool.tile([32, 1], I32, tag="ofsv")
                    nc.vector.tensor_copy(out=ofsv[:], in_=idxv[:])
                    voV = spool.tile([32, EMB], F32, tag="voV")
                    nc.gpsimd.indirect_dma_start(
                        out=voV[:], out_offset=None, in_=V[:],
                        in_offset=bass.IndirectOffsetOnAxis(ap=ofsv[:], axis=0),
                    )
                    # replicate V[vo[b]] to every flat row slot of each column
                    for c in range(1, NCOL + 1):
                        repV = rpool.tile([128, EMB], F32, tag=f"repV{c}")
                        nc.tensor.matmul(
                            out=repV[:],
                            lhsT=repmat_t[:, 128 * (c - 1):128 * c], rhs=voV[:],
                            start=True, stop=True,
                        )
                        repVs[c] = repV
                    continue

                c = (t + 3) // 4          # column 1..4
                m = (t - 1) % 4           # quarter-group within column
                if m == 0:
                    quad = qpool.tile([128, 8], F32, tag="quad")
                nc.tensor.matmul(
                    out=quad[:, 2 * m:2 * m + 2],
                    lhsT=foldq_t[:, 128 * m:128 * (m + 1)], rhs=vt[:],
                    start=True, stop=True,
                )
                if m != 3:
                    continue

                # -------- column c complete: extract idx, gather, dot ------
                pval = lpool.tile([128, 1], F32, tag="pval")
                nc.vector.tensor_reduce(
                    out=pval[:], in_=quad[:],
                    axis=mybir.AxisListType.X, op=mybir.AluOpType.add,
                )
                cnt = out_t[:, NCOL + c - 1:NCOL + c]
                nc.vector.tensor_scalar(
                    out=cnt, in0=pval[:], scalar1=MARK, scalar2=None,
                    op0=mybir.AluOpType.is_ge,
                )
                idxc = lpool.tile([128, 1], F32, tag="idxc")
                nc.vector.scalar_tensor_tensor(
                    out=idxc[:], in0=cnt, scalar=-MARK, in1=pval[:],
                    op0=mybir.AluOpType.mult, op1=mybir.AluOpType.add,
                )
                ofsc = lpool.tile([128, 1], I32, tag="ofsc")
                nc.vector.tensor_copy(out=ofsc[:], in_=idxc[:])
                rowU = gpool.tile([128, EMB], F32, tag="rowU")
                nc.gpsimd.indirect_dma_start(
                    out=rowU[:], out_offset=None, in_=U[:],
                    in_offset=bass.IndirectOffsetOnAxis(ap=ofsc[:], axis=0),
                )
                # fused: d[:, c-1] = sum(U_row * V_vo_row)
                nc.vector.scalar_tensor_tensor(
                    out=rowU[:], in0=rowU[:], scalar=1.0, in1=repVs[c][:],
                    op0=mybir.AluOpType.mult, op1=mybir.AluOpType.mult,
                    accum_out=out_t[:, c - 1:c],
                )

            nc.sync.dma_start(out=d_out[:], in_=out_t[:])

    if split_waits:
        _split_multi_waits(nc)
    return nc


def _consts():
    p = np.arange(128)
    qoff_np = ((p % 4) * QW).astype(np.float32).reshape(128, 1)
    foldq_np = np.zeros((128, 4 * 128), np.float32)
    for m in range(4):
        foldq_np[p, 128 * m + 32 * m + p // 4] = 1.0
    repmat_np = np.zeros((32, NCOL * 128), np.float32)
    for c in range(NCOL):
        for pp in range(128):
            f = 128 * c + pp
            b = f // CTX if f < NV else (f - NV) // K
            repmat_np[b, 128 * c + pp] = 1.0
    return qoff_np, foldq_np, repmat_np


_CACHE = {}


def kernel(vo, vi, neg_samples, V, U):
    if "nc" not in _CACHE:
        _CACHE["nc"] = _build()
        _CACHE["consts"] = _consts()
    nc = _CACHE["nc"]
    qoff_np, foldq_np, repmat_np = _CACHE["consts"]

    vo = np.ascontiguousarray(vo, dtype=np.float32)
    vi = np.ascontiguousarray(vi, dtype=np.float32)
    neg = np.ascontiguousarray(neg_samples, dtype=np.float32)
    V = np.ascontiguousarray(V, dtype=np.float32)
    U = np.ascontiguousarray(U, dtype=np.float32)

    in_maps = []
    for c in range(NCORES):
        sl = slice(c * BPC, (c + 1) * BPC)
        in_maps.append({
            "vo": vo[sl],
            "vi": vi[sl].reshape(NV, VOC),
            "ng": neg[sl].reshape(NN, VOC),
            "V": V,
            "U": U,
            "qoff": qoff_np,
            "foldq": foldq_np,
            "repmat": repmat_np,
        })

    res = run_bass_kernel_spmd(nc, in_maps, list(range(NCORES)))
    obs = []
    for r in res.results:
        o = r["out"]
        d_flat = o[:, 0:NCOL].flatten(order="F")      # flat vi+neg rows
        c_flat = o[:, NCOL:2 * NCOL].flatten(order="F")
        d_vi = d_flat[:NV].reshape(BPC, CTX)
        c_vi = c_flat[:NV].reshape(BPC, CTX)
        d_ng = d_flat[NV:NV + NN].reshape(BPC, K)
        lp = (d_vi * c_vi).sum(axis=1)
        ms = c_vi.sum(axis=1)
        x = lp / ms
        left = -np.log1p(np.exp(-x))
        right = (-np.log1p(np.exp(d_ng))).sum(axis=1)
        obs.append(-(left + right))
    ob = np.concatenate(obs)
    return np.float32(ob.mean(dtype=np.float64))


# revision 28
# speedup vs baseline: 1.1358x; 1.0334x over previous
"""CBOW negative-sampling loss kernel for 8 Trainium2 NeuronCores.

The reference computes one-hot @ table matmuls (embedding lookups in
disguise) followed by a tiny log-sigmoid loss.  Device-side algorithm:

Phase A (index extraction, streaming):
  Every one-hot row (50000 wide) is laid out as 4 partitions x 12500.
  The iota tile (value 65536 + (p%4)*12500 + j) is generated on-device
  (gpsimd iota + one DVE pass) instead of being DMAed from DRAM, so the
  HBM stream is exactly the one-hot bytes.  Each [128, 6250] chunk is
  consumed by ONE fused DVE tensor_tensor_reduce (mult + row-sum), and
  a per-quarter [128]->[128] fold matmul places each tile's 32 row
  values directly into a [128, col] PSUM layout (flat row = 128*col+p),
  eliminating the DRAM scratch round trips of the earlier version.

Phase B (gather + dots, pipelined per column):
  vo is tile 0 and owns psum column 0: its indices are ready early, so
  the 32 V rows are gathered and replicated to all 128 row slots of
  each column with tiny 0/1 matmuls on the (otherwise idle) tensor
  engine while streaming continues.  Each of the 4 vi/neg columns is
  finished as soon as its 4 extraction tiles land: idx = val - 65536*cnt,
  indirect-gather 128 U rows, fused DVE dot against the replicated V
  rows.  Only the last column's ~10us of work trails the final DMA.

Host: batch-shard across 8 cores, log-sigmoid loss terms + mean on CPU.
"""
import numpy as np

import concourse.bass as bass
import concourse.mybir as mybir
from concourse.tile import TileContext
from concourse.bass_utils import run_bass_kernel_spmd

VOC = 50000
EMB = 300
B = 256
CTX = 6
K = 10
NCORES = 8
BPC = B // NCORES                    # 32 batch rows per core
NV = BPC * CTX                       # 192 vi rows per core
NN = BPC * K                         # 320 neg rows per core
NTILES = 1 + (NV + NN) // 32         # 17 extraction tiles of [128, 12500]
QW = VOC // 4                        # 12500 per partition-quarter
CH = QW // 2                         # 6250 free-dim chunk
COLS = [4, 4, 4, 3, 1]               # vi+neg tiles per psum column
NCOL = len(COLS)                     # 5 columns: 128,128,128,96,32 rows
CVALID = [32 * n for n in COLS]      # valid partitions per column
MARK = 65536.0                       # cnt marker (> max idx, power of 2)
# last extraction tile streams as small chunks to shorten the serial tail
LAST_CHUNKS = [(0, 3125), (3125, 3125), (6250, 3125), (9375, 1563),
               (10938, 781), (11719, 781)]
# out_t column layout [d1 d2 d3 | cnt1..cnt4 | d4 d5 cnt5]: everything in
# cols 0..6 is complete when column 4 closes (ships mid-stream in one DMA);
# cols 7..9 complete in the tail (one final DMA).
DCOL = [0, 1, 2, 7, 8]               # d column of out_t, per psum column
CCOL = [3, 4, 5, 6, 9]               # cnt column of out_t, per psum column

F32 = mybir.dt.float32
I32 = mybir.dt.int32


def _split_multi_waits(nc):
    """This env's walrus accepts only ONE sync wait per instruction.
    Hoist extra waits into single-wait NoOps right before the owner."""
    cnt = 0
    for fn in nc.m.functions:
        for blk in fn.blocks:
            insts = list(blk.instructions)
            if not any(
                i.sync_info and i.sync_info.on_wait and len(i.sync_info.on_wait) > 1
                for i in insts
            ):
                continue
            new = []
            for inst in insts:
                si = inst.sync_info
                if si and si.on_wait and len(si.on_wait) > 1:
                    waits = list(si.on_wait)
                    for w in waits[:-1]:
                        cnt += 1
                        nop = mybir.InstNoOp(
                            name=f"mwsplit-{cnt}", engine=inst.engine, ins=[], outs=[]
                        )
                        nop.sync_info = mybir.SyncInfo(on_wait=[w], on_update=[])
                        new.append(nop)
                    inst.sync_info = mybir.SyncInfo(
                        on_wait=[waits[-1]], on_update=list(si.on_update or [])
                    )
                new.append(inst)
            blk.instructions = new
    return cnt


def _build(split_waits=True):
    nc = bass.Bass(enable_partition_id=False)

    vo = nc.declare_dram_parameter("vo", [BPC, VOC], F32, isOutput=False)
    vi = nc.declare_dram_parameter("vi", [NV, VOC], F32, isOutput=False)
    ng = nc.declare_dram_parameter("ng", [NN, VOC], F32, isOutput=False)
    V = nc.declare_dram_parameter("V", [VOC, EMB], F32, isOutput=False)
    U = nc.declare_dram_parameter("U", [VOC, EMB], F32, isOutput=False)
    qoff = nc.declare_dram_parameter("qoff", [128, 1], F32, isOutput=False)
    foldq = nc.declare_dram_parameter("foldq", [128, 4 * 128], F32, isOutput=False)
    repmat = nc.declare_dram_parameter("repmat", [32, NCOL * 128], F32, isOutput=False)
    d_out = nc.declare_dram_parameter("out", [128, 2 * NCOL], F32, isOutput=True)

    # per-tile [128, QW] sources: 4 partition-quarters per row.
    # tile 0 = vo (owns psum column 0); tiles 1..16 = vi(6) + neg(10),
    # grouped 4 tiles -> one 128-row column (flat row f = 128*(c-1) + p).
    srcs = [vo.rearrange("r (q f) -> (r q) f", q=4)]
    for u in range(CTX):
        srcs.append(vi[32 * u:32 * (u + 1), :].rearrange("r (q f) -> (r q) f", q=4))
    for u in range(K):
        srcs.append(ng[32 * u:32 * (u + 1), :].rearrange("r (q f) -> (r q) f", q=4))
    assert len(srcs) == NTILES

    with TileContext(nc) as tc:
        with (
            tc.tile_pool(name="const", bufs=1) as cpool,
            tc.tile_pool(name="data", bufs=4) as dpool,
            tc.tile_pool(name="vals", bufs=3) as vpool,
            tc.tile_pool(name="small", bufs=1) as spool,
            tc.tile_pool(name="col", bufs=2) as lpool,
            tc.tile_pool(name="gath", bufs=2) as gpool,
            tc.tile_pool(name="pcol", bufs=2, space="PSUM") as pcpool,
            tc.tile_pool(name="repp", bufs=2, space="PSUM") as rpool,
            tc.tile_pool(name="vops", bufs=1, space="PSUM") as opool,
        ):
            # ------- constants: ACT HWDGE ring, keeps SP ring streaming ----
            qoff_t = cpool.tile([128, 1], F32, tag="qoff")
            nc.scalar.dma_start(out=qoff_t[:], in_=qoff[:])
            foldq_t = cpool.tile([128, 4 * 128], F32, tag="foldq")
            nc.scalar.dma_start(out=foldq_t[:], in_=foldq[:])
            repmat_t = cpool.tile([32, NCOL * 128], F32, tag="repmat")
            nc.scalar.dma_start(out=repmat_t[:], in_=repmat[:])

            # iota[p, j] = 65536 + (p%4)*12500 + j, generated in halves so
            # the first chunk's DVE op can start ~15us sooner.
            iota_t = cpool.tile([128, QW], F32, tag="iota")
            for ih in range(2):
                sl = slice(ih * CH, (ih + 1) * CH)
                nc.gpsimd.iota(
                    out=iota_t[:, sl], pattern=[[1, CH]], base=int(MARK) + ih * CH,
                    channel_multiplier=0, allow_small_or_imprecise_dtypes=True,
                )
                nc.vector.tensor_scalar(
                    out=iota_t[:, sl], in0=iota_t[:, sl], scalar1=qoff_t[:, 0:1],
                    scalar2=None, op0=mybir.AluOpType.add,
                )

            out_t = spool.tile([128, 2 * NCOL], F32, tag="out_t")
            nc.vector.memset(out_t[:], 0.0)
            repVs = [None] * (NCOL + 1)

            # ---------------- streaming extraction + pipelined columns -----
            # The fold halves accumulate straight into a [128, 1] PSUM column
            # (chained tiny matmuls per column), so no DVE reduce is needed.
            # The last column is a single 32-row tile and the last tile
            # streams as small chunks, so the post-stream serial tail is a
            # 32-row gather + dot only.
            col_of_tile, m_of_tile, starts = [], [], []
            s = 1
            for ci, n in enumerate(COLS):
                starts.append(s)
                for mm in range(n):
                    col_of_tile.append(ci + 1)
                    m_of_tile.append(mm)
                s += n
            pcol = None
            pending = []              # (column, rowU tile) awaiting their dot
            for t in range(NTILES):
                if t == NTILES - 1:
                    chunks = LAST_CHUNKS
                else:
                    chunks = [(0, CH), (CH, CH)]
                if t == 0:
                    c, m, ntile = 0, 0, 1
                    pcol = opool.tile([128, 1], F32, tag="pvo")
                else:
                    c, m = col_of_tile[t - 1], m_of_tile[t - 1]
                    ntile = COLS[c - 1]
                    if m == 0:
                        pcol = pcpool.tile([128, 1], F32, tag="pcol")
                vt = vpool.tile([128, len(LAST_CHUNKS)], F32, tag="vt")
                for h, (off, csz) in enumerate(chunks):
                    chunk = dpool.tile([128, CH], F32, tag="chunk")
                    nc.sync.dma_start(
                        out=chunk[:, :csz], in_=srcs[t][:, off:off + csz]
                    )
                    # fused: prod = (chunk * 1) * iota ; vt[:,h] = sum(prod)
                    nc.vector.scalar_tensor_tensor(
                        out=chunk[:, :csz], in0=chunk[:, :csz], scalar=1.0,
                        in1=iota_t[:, off:off + csz],
                        op0=mybir.AluOpType.mult, op1=mybir.AluOpType.mult,
                        accum_out=vt[:, h:h + 1],
                    )
                    nc.tensor.matmul(
                        out=pcol[:], lhsT=foldq_t[:, 128 * m:128 * (m + 1)],
                        rhs=vt[:, h:h + 1],
                        start=(m == 0 and h == 0),
                        stop=(m == ntile - 1 and h == len(chunks) - 1),
                    )

                if t == 0:
                    # vo rows always valid: ofs = val - MARK (i32 cast out)
                    ofsv = spool.tile([32, 1], I32, tag="ofsv")
                    nc.vector.tensor_scalar(
                        out=ofsv[:], in0=pcol[0:32, :], scalar1=-MARK,
                        scalar2=None, op0=mybir.AluOpType.add,
                    )
                    voV = spool.tile([32, EMB], F32, tag="voV")
                    nc.gpsimd.indirect_dma_start(
                        out=voV[:], out_offset=None, in_=V[:],
                        in_offset=bass.IndirectOffsetOnAxis(ap=ofsv[:], axis=0),
                    )
                    continue
                if m != ntile - 1:
                    continue

                # -------- column c complete: extract idx, gather, dot ------
                vc = CVALID[c - 1]
                cc_ = CCOL[c - 1]
                cnt = out_t[:, cc_:cc_ + 1]
                nc.vector.tensor_scalar(
                    out=cnt, in0=pcol[:], scalar1=MARK, scalar2=None,
                    op0=mybir.AluOpType.is_ge,
                )
                ofsc = lpool.tile([128, 1], I32, tag="ofsc")
                nc.vector.scalar_tensor_tensor(
                    out=ofsc[:], in0=cnt, scalar=-MARK, in1=pcol[:],
                    op0=mybir.AluOpType.mult, op1=mybir.AluOpType.add,
                )
                if c == 1:
                    # replicate V[vo[b]] to every flat row slot per column;
                    # emitted here (after column 1's PSUM group closed) so
                    # accumulation groups never interleave on PE.  Each repV
                    # is copied to SBUF: the gpsimd dot cannot read PSUM.
                    for cc in range(1, NCOL + 1):
                        repP = rpool.tile([128, EMB], F32, tag="repP")
                        nc.tensor.matmul(
                            out=repP[:],
                            lhsT=repmat_t[:, 128 * (cc - 1):128 * cc],
                            rhs=voV[:], start=True, stop=True,
                        )
                        repV = cpool.tile([128, EMB], F32, tag=f"repV{cc}")
                        nc.scalar.activation(
                            out=repV[:], in_=repP[:],
                            func=mybir.ActivationFunctionType.Copy,
                        )
                        repVs[cc] = repV
                rowU = gpool.tile([128, EMB], F32, tag="rowU")
                nc.gpsimd.indirect_dma_start(
                    out=rowU[:vc, :], out_offset=None, in_=U[:],
                    in_offset=bass.IndirectOffsetOnAxis(ap=ofsc[:vc, :], axis=0),
                )
                # The fused dot d = sum(U_row * V_vo_row) runs on DVE, but
                # DEFERRED one column: column c's dot is emitted at column
                # c+1's close, ~70us after its gather landed, so the
                # in-order DVE stream never idles waiting for a gather.
                pending.append((c, rowU))
                if len(pending) > 1:
                    pc, prowU = pending.pop(0)
                    pvc, pdc = CVALID[pc - 1], DCOL[pc - 1]
                    # gate = (pcol >= -1) == all-ones, but DEPENDS on the
                    # current column's fold: the Tile scheduler provably
                    # cannot hoist the dot ahead of this point, so its
                    # gather has had a full column (~70us) to land.
                    gate = lpool.tile([128, 1], F32, tag="gate")
                    nc.vector.tensor_scalar(
                        out=gate[:], in0=pcol[:], scalar1=-1.0, scalar2=None,
                        op0=mybir.AluOpType.is_ge,
                    )
                    nc.vector.scalar_tensor_tensor(
                        out=prowU[:pvc, :], in0=prowU[:pvc, :],
                        scalar=gate[:pvc, :], in1=repVs[pc][:pvc, :],
                        op0=mybir.AluOpType.mult, op1=mybir.AluOpType.mult,
                        accum_out=out_t[:pvc, pdc:pdc + 1],
                    )
                if c == NCOL - 1:
                    # d1..d3 + cnt1..cnt4 ship while the last tile streams;
                    # ACT ring, so the SP chunk-DMA ring never waits on it.
                    nc.scalar.dma_start(out=d_out[:, 0:7], in_=out_t[:, 0:7])

            # final dots (columns 4 and 5) + the last three output columns
            for pc, prowU in pending:
                pvc, pdc = CVALID[pc - 1], DCOL[pc - 1]
                nc.vector.scalar_tensor_tensor(
                    out=prowU[:pvc, :], in0=prowU[:pvc, :], scalar=1.0,
                    in1=repVs[pc][:pvc, :],
                    op0=mybir.AluOpType.mult, op1=mybir.AluOpType.mult,
                    accum_out=out_t[:pvc, pdc:pdc + 1],
                )
            nc.scalar.dma_start(out=d_out[:, 7:10], in_=out_t[:, 7:10])

    if split_waits:
        _split_multi_waits(nc)
    return nc


def _col_starts():
    st, s = [], 0
    for n in COLS:
        st.append(s)
        s += 32 * n
    return st


def _consts():
    p = np.arange(128)
    qoff_np = ((p % 4) * QW).astype(np.float32).reshape(128, 1)
    foldq_np = np.zeros((128, 4 * 128), np.float32)
    for m in range(4):
        foldq_np[p, 128 * m + 32 * m + p // 4] = 1.0
    repmat_np = np.zeros((32, NCOL * 128), np.float32)
    starts = _col_starts()
    for c in range(NCOL):
        for pp in range(CVALID[c]):
            f = starts[c] + pp
            b = f // CTX if f < NV else (f - NV) // K
            repmat_np[b, 128 * c + pp] = 1.0
    return qoff_np, foldq_np, repmat_np


_CACHE = {}


def kernel(vo, vi, neg_samples, V, U):
    if "nc" not in _CACHE:
        _CACHE["nc"] = _build()
        _CACHE["consts"] = _consts()
    nc = _CACHE["nc"]
    qoff_np, foldq_np, repmat_np = _CACHE["consts"]

    vo = np.ascontiguousarray(vo, dtype=np.float32)
    vi = np.ascontiguousarray(vi, dtype=np.float32)
    neg = np.ascontiguousarray(neg_samples, dtype=np.float32)
    V = np.ascontiguousarray(V, dtype=np.float32)
    U = np.ascontiguousarray(U, dtype=np.float32)

    in_maps = []
    for c in range(NCORES):
        sl = slice(c * BPC, (c + 1) * BPC)
        in_maps.append({
            "vo": vo[sl],
            "vi": vi[sl].reshape(NV, VOC),
            "ng": neg[sl].reshape(NN, VOC),
            "V": V,
            "U": U,
            "qoff": qoff_np,
            "foldq": foldq_np,
            "repmat": repmat_np,
        })

    res = run_bass_kernel_spmd(nc, in_maps, list(range(NCORES)))
    obs = []
    for r in res.results:
        o = r["out"]
        d_flat = np.concatenate([o[:CVALID[c], DCOL[c]] for c in range(NCOL)])
        c_flat = np.concatenate([o[:CVALID[c], CCOL[c]] for c in range(NCOL)])
        d_vi = d_flat[:NV].reshape(BPC, CTX)
        c_vi = c_flat[:NV].reshape(BPC, CTX)
        d_ng = d_flat[NV:NV + NN].reshape(BPC, K)
        lp = (d_vi * c_vi).sum(axis=1)
        ms = c_vi.sum(axis=1)
        x = lp / ms
        left = -np.log1p(np.exp(-x))
        right = (-np.log1p(np.exp(d_ng))).sum(axis=1)
        obs.append(-(left + right))
    ob = np.concatenate(obs)
    return np.float32(ob.mean(dtype=np.float64))
